# revision 1
# baseline (speedup 1.0000x reference)
"""VQ Euclidean codebook kernel for Trainium2 (8 NeuronCores, SPMD data-parallel).

Computes, for x_td [65536,256] and embeddings_kd [2048,256] (fp32):
  dist = ||x||^2 - 2 x.e^T + ||e||^2 ; idx = argmin_k dist (first-index ties)
  quantized = x + (e[idx] - x)   (straight-through, fp32 two-step rounding)
Returns (quantized_td, indices_t) exactly like the reference nn.Module.

Sharding: x_td split along T across 8 cores; codebook replicated.
"""
import sys, os
sys.path.insert(0, '/opt/trn_rl_repo')
import numpy as np

import concourse.bass as bass
import concourse.bacc as bacc
import concourse.mybir as mybir
from concourse import bass_utils
from concourse.tile import TileContext

T_FULL = 65536
K = 2048
D = 256
P = 128
N_CORES = 8
T_SHARD = T_FULL // N_CORES          # 8192
NT = T_SHARD // P                    # 64 m-tiles per core
KT = K // P                          # 16 codebook tiles
DC = D // P                          # 2 contraction chunks

_CACHE = {}
LAST_RESULT = None


def _build():
    nc = bacc.Bacc('TRN2', target_bir_lowering=False, debug=False)
    f32 = mybir.dt.float32
    u32 = mybir.dt.uint32
    IDENT = mybir.ActivationFunctionType.Identity
    SQUARE = mybir.ActivationFunctionType.Square

    x_dram = nc.dram_tensor('x', (T_SHARD, D), f32, kind='ExternalInput')
    e_dram = nc.dram_tensor('e', (K, D), f32, kind='ExternalInput')
    id_dram = nc.dram_tensor('ident', (P, P), f32, kind='ExternalInput')
    q_dram = nc.dram_tensor('q', (T_SHARD, D), f32, kind='ExternalOutput')
    idx_dram = nc.dram_tensor('idx', (T_SHARD, 1), u32, kind='ExternalOutput')
    esq_scratch = nc.dram_tensor('esq_scratch', (K,), f32, kind='Internal')

    with TileContext(nc) as tc:
        with tc.tile_pool(name='const', bufs=1) as constp, \
             tc.tile_pool(name='etp', bufs=1) as etp, \
             tc.tile_pool(name='xall', bufs=1) as xallp, \
             tc.tile_pool(name='ld', bufs=3) as ldp, \
             tc.tile_pool(name='work', bufs=2) as workp, \
             tc.tile_pool(name='small', bufs=2) as smallp, \
             tc.tile_pool(name='trps', bufs=2, space='PSUM') as trps, \
             tc.tile_pool(name='cps', bufs=2, space='PSUM') as cps:

            ident_sb = constp.tile([P, P], f32)
            nc.sync.dma_start(out=ident_sb[:], in_=id_dram.ap())

            eT_sb = constp.tile([P, DC, K], f32)         # [d, dc, k] d-major codebook
            negesq = constp.tile([P, K], f32)            # -||e_k||^2 broadcast on partitions
            esq_cols = constp.tile([P, KT], f32)
            x_all = xallp.tile([P, NT * D], f32)         # all x rows (natural layout)
            xsq_all = constp.tile([P, NT], f32)          # ||x_m||^2 per tile column

            # ---------------- Phase 0: codebook prep ----------------
            for kt in range(KT):
                en = ldp.tile([P, D], f32, tag='en')
                nc.sync.dma_start(out=en[:], in_=e_dram.ap()[kt * P:(kt + 1) * P, :])
                sq = ldp.tile([P, D], f32, tag='esq_scr')
                nc.scalar.activation(out=sq[:], in_=en[:], func=SQUARE,
                                     accum_out=esq_cols[:, kt:kt + 1])
                for dc in range(DC):
                    tr = trps.tile([P, P], f32)
                    nc.tensor.transpose(tr[:], en[:, dc * P:(dc + 1) * P], ident_sb[:])
                    nc.scalar.activation(out=eT_sb[:, dc, kt * P:(kt + 1) * P], in_=tr[:],
                                         func=IDENT, bias=0.0, scale=1.0)
            # negate esq, scatter to DRAM, broadcast-load across partitions
            nc.scalar.activation(out=esq_cols[:], in_=esq_cols[:], func=IDENT,
                                 bias=0.0, scale=-1.0)
            nc.sync.dma_start(
                out=esq_scratch.ap().rearrange('(kt p) -> p kt', p=P),
                in_=esq_cols[:])
            nc.sync.dma_start(
                out=negesq[:],
                in_=esq_scratch.ap().rearrange('(o k) -> o k', o=1).to_broadcast([P, K]))

            # ---------------- Phase 1: load x, row sums of squares ----------------
            for nt in range(NT):
                xs = x_all[:, nt * D:(nt + 1) * D]
                nc.sync.dma_start(out=xs, in_=x_dram.ap()[nt * P:(nt + 1) * P, :])
                sq = ldp.tile([P, D], f32, tag='xsq_scr')
                nc.scalar.activation(out=sq[:], in_=xs, func=SQUARE,
                                     accum_out=xsq_all[:, nt:nt + 1])

            # ---------------- Phase 2: main loop ----------------
            for nt in range(NT):
                xs = x_all[:, nt * D:(nt + 1) * D]
                xT = workp.tile([P, DC, P], f32, tag='xT')
                for dc in range(DC):
                    tr = trps.tile([P, P], f32)
                    nc.tensor.transpose(tr[:], xs[:, dc * P:(dc + 1) * P], ident_sb[:])
                    nc.scalar.activation(out=xT[:, dc, :], in_=tr[:],
                                         func=IDENT, bias=0.0, scale=1.0)

                s1 = workp.tile([P, K], f32, tag='s1')
                s2n = workp.tile([P, K], f32, tag='s2n')
                for half in range(2):
                    c = cps.tile([P, 1024], f32)
                    for ns in range(2):
                        kb = half * 1024 + ns * 512
                        for dc in range(DC):
                            nc.tensor.matmul(
                                out=c[:, ns * 512:(ns + 1) * 512],
                                lhsT=xT[:, dc, :],
                                rhs=eT_sb[:, dc, kb:kb + 512],
                                start=(dc == 0), stop=(dc == DC - 1))
                    hs = slice(half * 1024, (half + 1) * 1024)
                    # s1 = fl(-2c + xsq)  (bitwise == reference t1)
                    nc.scalar.activation(out=s1[:, hs], in_=c[:], func=IDENT,
                                         bias=xsq_all[:, nt:nt + 1], scale=-2.0)
                    # s2n = fl(-esq - s1) = -fl(t1 + esq) = -dist
                    nc.gpsimd.tensor_tensor(out=s2n[:, hs], in0=negesq[:, hs],
                                            in1=s1[:, hs], op=mybir.AluOpType.subtract)

                m8 = smallp.tile([P, 8], f32, tag='m8')
                i8 = smallp.tile([P, 8], u32, tag='i8')
                nc.vector.max(out=m8[:], in_=s2n[:])
                nc.vector.max_index(out=i8[:], in_max=m8[:], in_values=s2n[:])
                nc.sync.dma_start(out=idx_dram.ap()[nt * P:(nt + 1) * P, :],
                                  in_=i8[:, 0:1])

                g = smallp.tile([P, D], f32, tag='g')
                nc.gpsimd.indirect_dma_start(
                    out=g[:], out_offset=None,
                    in_=e_dram.ap(),
                    in_offset=bass.IndirectOffsetOnAxis(ap=i8[:, 0:1], axis=0))
                # straight-through: out = fl(x + fl(g - x)) (bitwise == reference)
                d1 = smallp.tile([P, D], f32, tag='d1')
                oq = smallp.tile([P, D], f32, tag='oq')
                nc.vector.tensor_tensor(out=d1[:], in0=g[:], in1=xs,
                                        op=mybir.AluOpType.subtract)
                nc.vector.tensor_tensor(out=oq[:], in0=xs, in1=d1[:],
                                        op=mybir.AluOpType.add)
                nc.sync.dma_start(out=q_dram.ap()[nt * P:(nt + 1) * P, :], in_=oq[:])

    nc.compile()
    return nc


def kernel(x_td: np.ndarray, embeddings_kd: np.ndarray):
    global LAST_RESULT
    if 'nc' not in _CACHE:
        _CACHE['nc'] = _build()
    nc = _CACHE['nc']

    x_td = np.ascontiguousarray(x_td, dtype=np.float32)
    e = np.ascontiguousarray(embeddings_kd, dtype=np.float32)
    ident = np.eye(P, dtype=np.float32)

    in_maps = []
    for c in range(N_CORES):
        xs = x_td[c * T_SHARD:(c + 1) * T_SHARD]
        in_maps.append({'x': xs, 'e': e, 'ident': ident})

    res = bass_utils.run_bass_kernel_spmd(nc, in_maps, core_ids=list(range(N_CORES)))
    LAST_RESULT = res

    q = np.empty((T_FULL, D), dtype=np.float32)
    idx = np.empty((T_FULL,), dtype=np.int32)
    for c in range(N_CORES):
        q[c * T_SHARD:(c + 1) * T_SHARD] = res.results[c]['q']
        idx[c * T_SHARD:(c + 1) * T_SHARD] = res.results[c]['idx'].reshape(-1).view(np.int32)
    return q, idx
